# revision 27
# baseline (speedup 1.0000x reference)
"""Trainium2 Bass kernel for GNN mean-aggregation message passing.

  m = relu(concat(y[src], ex) @ W1.T + b1)        per edge
  z = segment_mean(m, dst)                        per node (0 for isolated)
  h = relu(z @ W2.T + b2)                         per node

Strategy (8 NeuronCores, one SPMD program):
  - Host shards edges by dst node range (N/8 nodes per core) and orders each
    core's edges by (psum-window, src-chunk, dst). Edge tiles are 128-edge
    matmul tiles; per-(window, chunk) tile counts and per-tile one-hot
    column spans are unioned across cores so a single program fits all
    shards (padding slots carry zero one-hot weight).
  - Per edge: m = relu(u[src] + (ex @ W1e.T + b1)), where u = y @ W1y.T is
    precomputed on device into a DRAM table [N, 64] (rows padded to 256 B),
    fetched row-contiguously by dma_gather (int16 indices -> the node table
    is processed in 32768-row chunks).
  - Scatter-sum on PE: s^T[48, win] += m[128e, 48].T @ O[128e, span], with
    O a 0/1 one-hot (exact in bf16). The mean is applied at window drain
    as an elementwise multiply with 1/deg (f32).
  - Node MLP: h^T[32, win] = relu(W2.T @ z^T + b2) streamed per window.
"""

import numpy as np
import ml_dtypes

N_CORES = 8
WIN = 1024         # nodes per PSUM scatter window (2 banks)
TILE_E = 128       # edges per scatter matmul (PE contraction dim)
SUPER = 8          # tiles per DVE/ACT add+relu batch
UCHUNK = 1024      # nodes per u-precompute chunk
CH = 32768         # u-table rows per gather chunk (int16 index limit)
UW = 64            # u-table row width (64 f32 = 256B, dma_gather elem size)

BF16 = ml_dtypes.bfloat16
DEBUG_OUTPUTS = False
DISABLE = set()  # bisection: subset of {"gather","exa","scatter","phaseA","w2"}


def _preprocess(y, ex, W1, b1, W2, b2, src, dst):
    N, ND = y.shape
    E, ED = ex.shape
    D = ND + ED
    NPC = N // N_CORES
    n_win = (NPC + WIN - 1) // WIN
    N_pad = ((N + UCHUNK - 1) // UCHUNK) * UCHUNK
    n_grp = (N_pad + CH - 1) // CH

    cnt = np.bincount(dst, minlength=N)
    inv_cnt = (1.0 / np.maximum(cnt, 1)).astype(np.float32)

    core_of = (dst // NPC).astype(np.int64)
    win_of = ((dst - core_of * NPC) // WIN).astype(np.int64)
    grp_of = (src // CH).astype(np.int64)
    NW, NG = n_win, n_grp
    cwg = (core_of * NW + win_of) * NG + grp_of
    key = cwg * np.int64(N + 1) + dst
    order = np.argsort(key, kind="stable")

    dst_s = dst[order].astype(np.int64)
    src_s = src[order].astype(np.int64)
    ex_s = ex[order]
    core_s = core_of[order]
    win_s = win_of[order]
    grp_s = grp_of[order]
    cwg_s = cwg[order]

    cwg_cnt = np.bincount(cwg_s, minlength=N_CORES * NW * NG).reshape(
        N_CORES, NW, NG)
    # unified tiles per (window, group) = max over cores
    T_wg = (cwg_cnt.max(axis=0) + TILE_E - 1) // TILE_E      # [NW, NG]
    # block layout: windows major, groups inside
    blocks_per_win = T_wg.sum(axis=1)                        # [NW]
    win_block_base = np.concatenate([[0], np.cumsum(blocks_per_win)])
    grp_block_off = np.concatenate(
        [np.zeros((NW, 1), np.int64), np.cumsum(T_wg, axis=1)[:, :-1]], axis=1)
    B_tot = int(win_block_base[-1])
    E_slots = B_tot * TILE_E

    # rank of each edge within its (core, window, group) run
    cwg_start = np.zeros(N_CORES * NW * NG + 1, np.int64)
    cwg_start[1:] = np.cumsum(cwg_cnt.reshape(-1))
    rank = np.arange(E, dtype=np.int64) - cwg_start[cwg_s]
    slot = (win_block_base[win_s] + grp_block_off[win_s, grp_s]) * TILE_E + rank
    tile_of = slot // TILE_E
    p_in_tile = slot % TILE_E

    # per-tile node span (relative to window start), unioned over cores
    rel = dst_s - core_s * NPC - win_s * WIN
    lo_t = np.full(B_tot, np.int64(1 << 60))
    hi_t = np.full(B_tot, np.int64(-1))
    np.minimum.at(lo_t, tile_of, rel)
    np.maximum.at(hi_t, tile_of, rel)
    empty = hi_t < 0
    lo_t[empty] = 0
    hi_t[empty] = 0
    span_t = hi_t - lo_t + 1
    col_off = np.concatenate([[0], np.cumsum(span_t)])
    C_tot = int(col_off[-1])
    o_col = col_off[tile_of] + (rel - lo_t[tile_of])

    # gather index arrays: per (w,g) flat list of 128*T_wg chunk-local idxs,
    # wrapped as [128, n/16] int16 (16-partition wrap, replicated 8x)
    IC = E_slots // 16
    idx_flat = np.zeros((N_CORES, E_slots), np.int16)
    loc = (src_s - grp_s * CH).astype(np.int16)
    for c in range(N_CORES):
        m = core_s == c
        idx_flat[c, slot[m]] = loc[m]
    idx_wrap = np.zeros((N_CORES, 128, IC), np.int16)
    for c in range(N_CORES):
        w16 = idx_flat[c].reshape(IC, 16).T  # [16, IC]
        idx_wrap[c] = np.tile(w16, (8, 1))

    exT1 = np.zeros((N_CORES, ED + 1, E_slots), BF16)
    O_a = np.zeros((N_CORES, TILE_E, C_tot), BF16)
    for c in range(N_CORES):
        m = core_s == c
        sl = slot[m]
        exT1[c, :ED, sl] = ex_s[m].astype(BF16)
        exT1[c, ED, sl] = 1.0
        O_a[c, p_in_tile[m], o_col[m]] = 1.0

    cinv = np.empty((N_CORES, ND + ED, NPC), np.float32)
    for c in range(N_CORES):
        cinv[c] = np.broadcast_to(inv_cnt[c * NPC : (c + 1) * NPC], (D, NPC))

    yT = np.ascontiguousarray(y.T).astype(BF16)
    if N_pad != N:
        yT = np.concatenate([yT, np.zeros((ND, N_pad - N), BF16)], 1)

    meta = {
        "N": N, "E": E, "ND": ND, "ED": ED, "D": D, "NPC": NPC,
        "n_win": NW, "n_grp": NG, "N_pad": N_pad,
        "T_wg": T_wg, "win_block_base": win_block_base,
        "grp_block_off": grp_block_off, "B_tot": B_tot, "E_slots": E_slots,
        "C_tot": C_tot, "lo_t": lo_t, "span_t": span_t, "col_off": col_off,
    }
    consts = dict(
        yT=yT,
        W1y=np.ascontiguousarray(W1[:, :ND].T).astype(BF16),
        W1eb=np.concatenate([W1[:, ND:].T, b1[None, :]], 0).astype(BF16),
        W2b=np.ascontiguousarray(W2.T).astype(np.float32),
        b2=np.ascontiguousarray(b2.reshape(-1, 1)).astype(np.float32),
    )
    per_core = dict(exT1=exT1, idx=idx_wrap, O=O_a, cinv=cinv)
    return consts, per_core, meta


def _split_excess_waits(nc, mybir):
    """This walrus build accepts at most 1 sync wait per instruction (0 on
    Drain). Move extras onto NOPs inserted just before, same engine."""
    for fn in nc.m.functions:
        for bb in fn.blocks:
            new_list = []
            for ins in bb.instructions:
                si = ins.sync_info
                limit = 0 if isinstance(ins, mybir.InstDrain) else 1
                if si is not None and si.on_wait and len(si.on_wait) > limit:
                    waits = list(si.on_wait)
                    keep, extra = waits[:limit], waits[limit:]
                    while extra:
                        chunk, extra = extra[:1], extra[1:]
                        nop = mybir.InstNoOp(
                            name=nc.get_next_instruction_name(), ins=[], outs=[])
                        nop.engine = ins.engine
                        nop.sync_info = mybir.SyncInfo(on_wait=chunk, on_update=[])
                        nc.register_instruction(nop)
                        new_list.append(nop)
                    si.on_wait = keep
                new_list.append(ins)
            bb.instructions[:] = new_list


def _build_program(meta):
    import concourse.bacc as bacc
    import concourse.mybir as mybir
    import concourse.tile as tile
    from concourse import library_config

    f32 = mybir.dt.float32
    bf16 = mybir.dt.bfloat16
    i16 = mybir.dt.int16
    Relu = mybir.ActivationFunctionType.Relu
    Copy = mybir.ActivationFunctionType.Copy
    ADD = mybir.AluOpType.add
    MULT = mybir.AluOpType.mult

    N, ND, ED, D = meta["N"], meta["ND"], meta["ED"], meta["D"]
    NPC, NW, NG = meta["NPC"], meta["n_win"], meta["n_grp"]
    N_pad = meta["N_pad"]
    T_wg, wbb = meta["T_wg"], meta["win_block_base"]
    gbo = meta["grp_block_off"]
    B_tot, E_slots, C_tot = meta["B_tot"], meta["E_slots"], meta["C_tot"]
    lo_t, span_t, col_off = meta["lo_t"], meta["span_t"], meta["col_off"]
    IC = E_slots // 16
    OD = 32

    nc = bacc.Bacc("TRN2")
    yT_ext = nc.dram_tensor("yT", [ND, N_pad], bf16, kind="ExternalInput")
    ex_ext = nc.dram_tensor("exT1", [ED + 1, E_slots], bf16, kind="ExternalInput")
    idx_ext = nc.dram_tensor("idx", [128, IC], i16, kind="ExternalInput")
    O_ext = nc.dram_tensor("Omat", [TILE_E, C_tot], bf16, kind="ExternalInput")
    cinv_ext = nc.dram_tensor("cinv", [D, NPC], f32, kind="ExternalInput")
    w1y_ext = nc.dram_tensor("W1y", [ND, D], bf16, kind="ExternalInput")
    w1eb_ext = nc.dram_tensor("W1eb", [ED + 1, D], bf16, kind="ExternalInput")
    w2b_ext = nc.dram_tensor("W2b", [D, OD], f32, kind="ExternalInput")
    b2_ext = nc.dram_tensor("b2", [OD, 1], f32, kind="ExternalInput")
    out_ext = nc.dram_tensor("hT", [OD, NPC], f32, kind="ExternalOutput")
    if DEBUG_OUTPUTS:
        u_ext = nc.dram_tensor("u_dbg", [N_pad, UW], f32, kind="ExternalOutput")
        z_ext = nc.dram_tensor("z_dbg", [D, NPC], f32, kind="ExternalOutput")

    with tile.TileContext(nc) as tc:
        with (
            tc.tile_pool(name="const", bufs=1) as cpool,
            tc.tile_pool(name="dram", bufs=1, space="DRAM") as dpool,
            tc.tile_pool(name="io", bufs=2) as iopool,
            tc.tile_pool(name="gath", bufs=2) as gpool,
            tc.tile_pool(name="msb", bufs=3) as mpool,
            tc.tile_pool(name="psA", bufs=2, space="PSUM") as psA,
            tc.tile_pool(name="psZ", bufs=2, space="PSUM") as psZ,
            tc.tile_pool(name="psH", bufs=1, space="PSUM") as psH,
        ):
            nc.gpsimd.load_library(library_config.mlp)
            u_dram = dpool.tile([N_pad, UW], f32)

            w1y_sb = cpool.tile([ND, D], bf16)
            nc.sync.dma_start(out=w1y_sb[:], in_=w1y_ext[:])
            w1eb_sb = cpool.tile([ED + 1, D], bf16)
            nc.sync.dma_start(out=w1eb_sb[:], in_=w1eb_ext[:])
            w2b_sb = cpool.tile([D, OD], f32)
            nc.sync.dma_start(out=w2b_sb[:], in_=w2b_ext[:])
            b2_sb = cpool.tile([OD, 1], f32)
            nc.sync.dma_start(out=b2_sb[:], in_=b2_ext[:])
            zl_bf = cpool.tile([1, D], bf16)
            nc.any.memset(zl_bf[:], 0)
            zr_bf = cpool.tile([1, 512], bf16)
            nc.any.memset(zr_bf[:], 0)

            # phase A: u = y @ W1y.T -> DRAM table [N_pad, 64] (48 used)
            nsub = UCHUNK // TILE_E
            half = (nsub // 2) * D
            us01 = []
            for j in range(2):
                us = cpool.tile([TILE_E, nsub, UW], f32, tag=f"us{j}")
                nc.any.memset(us[:], 0)
                us01.append(us)
            n_chunks = 0 if "phaseA" in DISABLE else N_pad // UCHUNK
            for ci in range(n_chunks):
                c0 = ci * UCHUNK
                yt = iopool.tile([ND, UCHUNK], bf16, tag="yt")
                nc.scalar.dma_start(out=yt[:], in_=yT_ext[:, c0 : c0 + UCHUNK])
                ps = psA.tile([TILE_E, half], f32, tag="psa")
                ps2 = psA.tile([TILE_E, half], f32, tag="psa")
                for s in range(nsub):
                    dps, sd = (ps, s) if s < nsub // 2 else (ps2, s - nsub // 2)
                    nc.tensor.matmul(
                        dps[:, sd * D : (sd + 1) * D],
                        yt[:, s * TILE_E : (s + 1) * TILE_E],
                        w1y_sb[:], start=True, stop=True)
                us = us01[ci % 2]
                nc.vector.tensor_copy(
                    out=us[:, : nsub // 2, :D],
                    in_=ps[:].rearrange("p (b d) -> p b d", d=D))
                nc.scalar.activation(
                    out=us[:, nsub // 2 :, :D],
                    in_=ps2[:].rearrange("p (b d) -> p b d", d=D), func=Copy)
                out_ap = u_dram[c0 : c0 + UCHUNK, :].rearrange(
                    "(b p) d -> p b d", p=TILE_E)
                nc.sync.dma_start(out=out_ap, in_=us[:])
                if DEBUG_OUTPUTS:
                    nc.sync.dma_start(
                        out=u_ext[c0 : c0 + UCHUNK, :].rearrange(
                            "(b p) d -> p b d", p=TILE_E), in_=us[:])

            # phase B: edge pipeline, double-buffered per (window, src-chunk)
            for w in range(NW):
                B = int(wbb[w + 1] - wbb[w])
                if B == 0:
                    continue
                wn = min(WIN, NPC - w * WIN)
                b0 = int(wbb[w])

                cinv_t = gpool.tile([D, WIN], f32, tag="cinv")
                nc.sync.dma_start(
                    out=cinv_t[:, :wn], in_=cinv_ext[:, w * WIN : w * WIN + wn])

                psz = psZ.tile([D, WIN], f32, tag="psz")
                for j in range(0, WIN, 512):
                    nc.tensor.matmul(
                        psz[:, j : j + 512], zl_bf[:], zr_bf[:],
                        start=True, stop=True)

                groups = [g for g in range(NG) if int(T_wg[w, g]) > 0]
                for gi, g in enumerate(groups):
                    T = int(T_wg[w, g])
                    gb0 = b0 + int(gbo[w, g])       # first block of group
                    ge0 = gb0 * TILE_E              # first edge slot
                    gn = T * TILE_E

                    ex_t = gpool.tile([ED + 1, T * TILE_E], bf16, tag="ext")
                    nc.sync.dma_start(
                        out=ex_t[:], in_=ex_ext[:, ge0 : ge0 + gn])
                    csp = int(col_off[gb0 + T] - col_off[gb0])
                    o_t = gpool.tile([TILE_E, csp], bf16, tag="omat")
                    nc.sync.dma_start(
                        out=o_t[:],
                        in_=O_ext[:, int(col_off[gb0]) : int(col_off[gb0]) + csp])
                    ug = gpool.tile([TILE_E, T, UW], f32, tag="ug")
                    if "gather" in DISABLE:
                        T_rd = min(T, N_pad // TILE_E)
                        nc.sync.dma_start(
                            out=ug[:, :T_rd, :],
                            in_=u_dram[: T_rd * TILE_E, :].rearrange(
                                "(b p) d -> p b d", p=TILE_E))
                        if T_rd < T:
                            nc.vector.tensor_copy(
                                out=ug[:, T_rd:, :], in_=ug[:, : T - T_rd, :])
                    else:
                        idx_t = gpool.tile([128, gn // 16], i16, tag="idx")
                        nc.sync.dma_start(
                            out=idx_t[:],
                            in_=idx_ext[:, ge0 // 16 : ge0 // 16 + gn // 16])
                        rows = min(CH, N_pad - g * CH)
                        nc.gpsimd.dma_gather(
                            out_ap=ug[:, :, :],
                            in_ap=u_dram[g * CH : g * CH + rows, :],
                            idxs_ap=idx_t[:, :],
                            num_idxs=gn,
                            num_idxs_reg=gn,
                            elem_size=UW,
                            single_packet=False,
                        )

                    n_super = (T + SUPER - 1) // SUPER
                    for si in range(n_super):
                        s = si * SUPER
                        sb = min(SUPER, T - s)
                        m_sb = mpool.tile([TILE_E, SUPER * D], bf16, tag="m")
                        if "exa" in DISABLE:
                            nc.vector.tensor_copy(
                                out=m_sb[:, : sb * D].rearrange(
                                    "p (b d) -> p b d", d=D),
                                in_=ug[:, s : s + sb, :D])
                        else:
                            ps_a = psA.tile([TILE_E, SUPER * D], f32, tag="psa")
                            for t in range(sb):
                                nc.tensor.matmul(
                                    ps_a[:, t * D : (t + 1) * D],
                                    ex_t[:, (s + t) * TILE_E
                                         : (s + t + 1) * TILE_E],
                                    w1eb_sb[:], start=True, stop=True)
                            nc.vector.tensor_tensor(
                                out=m_sb[:, : sb * D].rearrange(
                                    "p (b d) -> p b d", d=D),
                                in0=ps_a[:, : sb * D].rearrange(
                                    "p (b d) -> p b d", d=D),
                                in1=ug[:, s : s + sb, :D],
                                op=ADD)
                        nc.scalar.activation(
                            out=m_sb[:, : sb * D], in_=m_sb[:, : sb * D],
                            func=Relu)
                        for t in range(sb if "scatter" not in DISABLE else 0):
                            bt = gb0 + s + t
                            lo = int(lo_t[bt])
                            sp = int(span_t[bt])
                            off = int(col_off[bt] - col_off[gb0])
                            last = (gi == len(groups) - 1
                                    and si == n_super - 1 and t == sb - 1)
                            # split at 512-col PSUM bank boundary
                            cuts = [lo, sp]
                            if lo // 512 != (lo + sp - 1) // 512:
                                sp1 = (lo // 512 + 1) * 512 - lo
                                cuts = [lo, sp1, lo + sp1, sp - sp1]
                            for k in range(0, len(cuts), 2):
                                clo, csp2 = cuts[k], cuts[k + 1]
                                if csp2 <= 0:
                                    continue
                                nc.tensor.matmul(
                                    psz[:, clo : clo + csp2],
                                    m_sb[:, t * D : (t + 1) * D],
                                    o_t[:, off + (clo - lo)
                                        : off + (clo - lo) + csp2],
                                    start=False,
                                    stop=last and k + 2 >= len(cuts),
                                    skip_group_check=True)

                zt = mpool.tile([D, WIN], f32, tag="zt")
                nc.vector.tensor_tensor(
                    out=zt[:, :wn], in0=psz[:, :wn], in1=cinv_t[:, :wn], op=MULT)
                if DEBUG_OUTPUTS:
                    nc.sync.dma_start(
                        out=z_ext[:, w * WIN : w * WIN + wn], in_=zt[:, :wn])
                ps_h = psH.tile([OD, WIN], f32, tag="psh")
                for j in range(0, wn, 512):
                    jn = min(512, wn - j)
                    nc.tensor.matmul(
                        ps_h[:, j : j + jn], w2b_sb[:], zt[:, j : j + jn],
                        start=True, stop=True)
                h_sb = mpool.tile([OD, WIN], f32, tag="h")
                nc.scalar.activation(
                    out=h_sb[:, :wn], in_=ps_h[:, :wn], func=Relu,
                    bias=b2_sb[:, 0:1])
                nc.sync.dma_start(
                    out=out_ext[:, w * WIN : w * WIN + wn], in_=h_sb[:, :wn])

    nc.compile()
    _split_excess_waits(nc, mybir)
    return nc


def kernel(y, ex, W1, b1, W2, b2, src, dst):
    from concourse.bass_utils import run_bass_kernel_spmd

    y = np.asarray(y, dtype=np.float32)
    ex = np.asarray(ex, dtype=np.float32)
    W1 = np.asarray(W1, dtype=np.float32)
    b1 = np.asarray(b1, dtype=np.float32)
    W2 = np.asarray(W2, dtype=np.float32)
    b2 = np.asarray(b2, dtype=np.float32)
    src = np.asarray(src, dtype=np.int32)
    dst = np.asarray(dst, dtype=np.int32)

    consts, per_core, meta = _preprocess(y, ex, W1, b1, W2, b2, src, dst)
    nc = _build_program(meta)

    in_maps = []
    for c in range(N_CORES):
        in_maps.append({
            "yT": consts["yT"],
            "exT1": per_core["exT1"][c],
            "idx": per_core["idx"][c],
            "Omat": per_core["O"][c],
            "cinv": per_core["cinv"][c],
            "W1y": consts["W1y"],
            "W1eb": consts["W1eb"],
            "W2b": consts["W2b"],
            "b2": consts["b2"],
        })
    res = run_bass_kernel_spmd(nc, in_maps, list(range(N_CORES)))

    NPC = meta["NPC"]
    h = np.empty((meta["N"], 32), dtype=np.float32)
    for c in range(N_CORES):
        h[c * NPC : (c + 1) * NPC, :] = res.results[c]["hT"].T
    if DEBUG_OUTPUTS:
        return h, res.results, meta
    return h
